# revision 11
# baseline (speedup 1.0000x reference)
"""Trainium2 Bass kernel for the didgeridoo (conical bore) input-impedance
model.

Math (matches the reference to first order in the tiny wall-loss alpha,
~1e-7 rel): chain of 128 lossy transmission-line 2x2 complex matrices per
frequency, Ze = (A*ZL + B)/(C*ZL + D), output |Ze|.

v2 strategy:
- 48 freqs/core; partition p = 32q + 16h + j holds freq 16q+j and slice-half
  h (64 slices each) -> 96 partitions active, every tree level half the
  baseline width.
- Level-1 (pair) matrices built DIRECTLY from hyperbolic product identities
  (exact in the angle, first-order in alpha): kills the level-0 build and
  the widest tree level.
- Slice pairs stored in bit-reversed order so every tree level multiplies
  contiguous left/right halves (combine i with i+n/2).
- Per level: one ACT negate of the left-half imag planes, 8 tensor-tensor
  multiplies (4 DVE / 4 GpSimd) writing contiguous term blocks, and ONE
  merged tensor_reduce producing the whole next level.
- After 5 in-half levels, stream_shuffle moves half-1 products onto half-0
  partitions; the radiation load ZL folds into the shuffled operand and one
  final combine + |A/C| produces the output.
- The constant t-grid loads once outside the loop; per-tick DMA is 4 cols.
- Timing build uses For_i_pipelined(unroll=16, staggered_reset=True,
  branch-prefetch hints) so consecutive evaluations overlap (no all-engine
  barrier or IRAM miss at the back edge).
"""
import math
from contextlib import ExitStack

import numpy as np

import concourse.bass as bass
import concourse.bacc as bacc
import concourse.tile as tile
from concourse import mybir
from concourse.bass_utils import run_bass_kernel_spmd

RHO = 1.2929
C_SOUND = 343.37
N_SUB = 128
N_CORES = 8
D0 = 32.0
NPAIR = 32          # pairs per half
P = 96              # active partitions: 3 quadrants x (16 freqs x 2 halves)
FPC = 48            # frequencies per core

F32 = mybir.dt.float32
MULT = mybir.AluOpType.mult
ADD = mybir.AluOpType.add
SUB = mybir.AluOpType.subtract
IDENT = mybir.ActivationFunctionType.Identity
COPY = mybir.ActivationFunctionType.Copy
SQUARE = mybir.ActivationFunctionType.Square
SQRT = mybir.ActivationFunctionType.Sqrt
X = mybir.AxisListType.X

Z0C = RHO * C_SOUND / math.pi   # z0 = Z0C * rinv^2
NEG_ON_ACT = True               # emit the per-level imag-negate on ACT vs DVE
OUT_DMA_ON_ACT = False          # issue the output DMA from ACT instead of SP
HINT_ENGINES = ("DVE", "Pool", "Activation")  # branch-prefetch hint engines
STAGED_BUFS = None              # intermediate-tile copies (None -> = unroll)
SHORT_BUFS = None               # buffer copies for short-lived tiles
SPLIT_REDUCE = False            # split per-level reduce (worse: +5 instrs beat the stall saving)
SWAP_IMIM = False               # put negate-dependent imim mults on GpSimd
TWO_STAGES = False              # split head/tree into 2 pipeline stages
                                # (trips a buffer-rebind race in the sim; off)


def _bitrev5(i):
    r = 0
    for _ in range(5):
        r = (r << 1) | (i & 1)
        i >>= 1
    return r


def _ap(t, off, dims):
    """AP over tile t's handle with partition dim first."""
    a = t[:]
    return bass.AP(a.tensor, off, [[a.ap[0][0], P]] + dims)


def _emit_head(nc, T, xd, tgt):
    """Stage 1: DMA in -> prep -> level-1 build. Returns (pc1, zl tiles).

    tgt: persistent [P,64] t-grid tile (pure constant, loaded once)."""
    V, G, S = nc.vector, nc.gpsimd, nc.scalar
    NP = NPAIR

    x = T(4, "x")                     # [f, sqrtf, len, d1]
    nc.sync.dma_start(out=x[:], in_=xd.ap()[:, 0:4])
    f = x[:, 0:1]
    sqf = x[:, 1:2]
    ln = x[:, 2:3]
    d1 = x[:, 3:4]
    tg = tgt[:, 0:64]

    # ---- per-frequency scalars [P,1] ----
    yp = T(1, "yp")      # 2*k*dL
    V.scalar_tensor_tensor(yp[:], f, 4.0 * math.pi / C_SOUND * (10.0 / 1000.0 / N_SUB), ln, MULT, MULT)
    dls = T(1, "dls")    # 3e-5*dL
    S.activation(dls[:], ln, COPY, scale=3e-5 * (10.0 / 1000.0 / N_SUB))
    sf = T(1, "sf")      # 3e-5*sqrt(f)*dL
    S.activation(sf[:], sqf, COPY, scale=dls[:])
    yp2 = T(1, "yp2")
    S.activation(yp2[:], yp[:], SQUARE)
    cyh = T(1, "cyh")
    S.activation(cyh[:], yp2[:], IDENT, scale=1.0 / 24.0, bias=-0.5)
    c2t = T(1, "c2")
    S.activation(c2t[:], cyh[:], IDENT, scale=yp2[:], bias=1.0)
    c2 = c2t[:]
    syh = T(1, "syh")
    S.activation(syh[:], yp2[:], IDENT, scale=1.0 / 120.0, bias=-1.0 / 6.0)
    syw = T(1, "syw")
    S.activation(syw[:], syh[:], IDENT, scale=yp2[:], bias=1.0)
    s2 = T(1, "s2")
    S.activation(s2[:], syw[:], COPY, scale=yp[:])
    qss = T(1, "qss")
    S.activation(qss[:], sf[:], COPY, scale=s2[:])
    qsc = T(1, "qsc")
    S.activation(qsc[:], sf[:], COPY, scale=c2)

    # radiation impedance ZL [P,1]
    kr = T(1, "kr")
    V.scalar_tensor_tensor(kr[:], f, 2.0 * math.pi / C_SOUND / 2000.0, d1, MULT, MULT)
    d1i = T(1, "d1i")
    V.reciprocal(d1i[:], d1)
    z0e = T(1, "z0e")
    V.scalar_tensor_tensor(z0e[:], d1i[:], Z0C * 2000.0 * 2000.0, d1i[:], MULT, MULT)
    kr2 = T(1, "kr2")
    S.activation(kr2[:], kr[:], SQUARE)
    zlre = T(1, "zlre", short=False)
    V.scalar_tensor_tensor(zlre[:], kr2[:], 0.25, z0e[:], MULT, MULT)
    zlim = T(1, "zlim", short=False)
    V.scalar_tensor_tensor(zlim[:], kr[:], 0.61, z0e[:], MULT, MULT)
    zlimn = T(1, "zlimn", short=False)
    S.activation(zlimn[:], zlim[:], COPY, scale=-1.0)
    dd = T(1, "dd")
    S.activation(dd[:], d1, IDENT, scale=1.0 / 2000.0, bias=-D0 / 2000.0)

    # ---- slice vectors [P,64] ----
    rv = T(64, "rv")
    S.activation(rv[:], tg, IDENT, scale=dd[:], bias=D0 / 2000.0)
    rinv = T(64, "rinv")
    V.reciprocal(rinv[:], rv[:])

    # strided even/odd views [P,32]
    def ev(t):
        return _ap(t, 0, [[2, NP]])

    def od(t):
        return _ap(t, 1, [[2, NP]])

    def dup(t, base=0):          # [P,32] half duplicated to [P, 2x32]
        return _ap(t, base, [[0, 2], [1, NP]])

    # ---- pair constants, packed in [P,64] sibling-pair tiles ----
    # QQ = (r_o*rinv_e | r_e*rinv_o) via one negative-stride TT
    qq = T(2 * NP, "qq")
    V.tensor_tensor(qq[:],
                    _ap(rv, 1, [[-1, 2], [2, NP]]),
                    _ap(rinv, 0, [[1, 2], [2, NP]]), MULT)
    rrp = T(2 * NP, "rrp")       # (Z0e/Z0o | Z0o/Z0e)
    S.activation(rrp[:], qq[:], SQUARE)
    uu = T(2 * NP, "uu")         # (u | ut)
    S.activation(uu[:], rrp[:], IDENT, scale=0.5, bias=0.5)
    vv = T(2 * NP, "vv")         # (v | vt)
    S.activation(vv[:], rrp[:], IDENT, scale=-0.5, bias=0.5)
    sa = T(NP, "sa")
    V.tensor_add(sa[:], ev(rinv), od(rinv))
    sd2 = T(2 * NP, "sd2")       # (-sd | +sd)
    G.tensor_sub(sd2[:, 0:NP], od(rinv), ev(rinv))
    G.tensor_sub(sd2[:, NP:2 * NP], ev(rinv), od(rinv))
    q4 = T(NP, "q4")
    G.tensor_mul(q4[:], ev(rv), ev(rv))        # r_e^2
    zz = T(2 * NP, "zz")         # (Z0_o | 1/Z0_e)
    V.scalar_tensor_tensor(zz[:, 0:NP], od(rinv), Z0C, od(rinv), MULT, MULT)
    G.tensor_scalar(zz[:, NP:2 * NP], q4[:], math.pi / (RHO * C_SOUND), None, MULT)
    usa = T(2 * NP, "usa")       # (u*sa | ut*sa)
    V.tensor_tensor(usa[:], uu[:], dup(sa), MULT)
    pbc = T(2 * NP, "pbc")       # (Z0o*u | u/Z0e)
    G.tensor_tensor(pbc[:], zz[:], dup(uu), MULT)
    pbcr1 = T(2 * NP, "pbcr1")   # pbc*sa
    V.tensor_tensor(pbcr1[:], pbc[:], dup(sa), MULT)
    tt13 = T(2 * NP, "tt13")     # (-Z0o*sd | sd/Z0e)
    G.tensor_tensor(tt13[:], zz[:], sd2[:], MULT)
    pbcr2 = T(2 * NP, "pbcr2")   # tt13*v
    V.tensor_tensor(pbcr2[:], tt13[:], dup(vv), MULT)

    # ---- level-1 build: pc [P, 8*32] = [Are|Bre|Cre|Dre|Aim|Bim|Cim|Dim] ----
    n = NP
    pc = T(8 * n, "pc1", short=False)
    # A_re & D_re in one op (plane stride 3n), likewise the other sibling pairs
    V.scalar_tensor_tensor(_ap(pc, 0, [[3 * n, 2], [1, n]]),
                           uu[:], c2, vv[:], MULT, ADD)
    S.activation(_ap(pc, 4 * n, [[3 * n, 2], [1, n]]),
                 usa[:], COPY, scale=qss[:])                       # A_im & D_im
    S.activation(_ap(pc, 5 * n, [[n, 2], [1, n]]),
                 pbc[:], COPY, scale=s2[:])                        # B_im & C_im
    tmpbc = T(2 * n, "tmpbc")
    S.activation(tmpbc[:], pbcr1[:], COPY, scale=qsc[:])
    V.scalar_tensor_tensor(pc[:, n:3 * n], pbcr2[:], sf[:], tmpbc[:], MULT, ADD)  # B_re & C_re

    return pc, zlre, zlim, zlimn


def _emit_tree_tail(nc, T, handoff, outd):
    """Stage 2: tree levels -> shuffle -> ZL fold -> final combine -> DMA."""
    V, G, S = nc.vector, nc.gpsimd, nc.scalar
    pc, zlre, zlim, zlimn = handoff
    n = NPAIR

    # ---- tree: combine i with i+n/2; contiguous left/right halves ----
    # u-tile term layout: element (c,t,e,p) at c*16m + t*4m + e*m + p, so
    # every multiply writes contiguously and ONE merged reduce (X over the
    # strided t-dim) produces the whole next level.
    lvl = 0
    while n > 1:
        m = n // 2
        lvl += 1
        # nim[e*m+p] = -L.im(e,p): left-half imag planes negated (V, so it
        # chains right behind the reduce that produced pc's imag half)
        nim = T(4 * m, f"nim{lvl}")
        if NEG_ON_ACT:
            S.activation(nim[:], _ap(pc, 4 * n, [[n, 4], [1, m]]),
                         COPY, scale=-1.0)
        else:
            V.tensor_scalar(nim[:], _ap(pc, 4 * n, [[n, 4], [1, m]]),
                            -1.0, None, MULT)
        u_t = T(32 * m, f"u{lvl}")
        q = T(8 * m, f"pc{lvl}")

        if m > 1:
            def lap(part, j):
                return _ap(pc, part * 4 * n + j * n, [[2 * n, 2], [0, 2], [1, m]])

            def nlap(j):
                return _ap(nim, j * m, [[2 * m, 2], [0, 2], [1, m]])

            def rap(part, j):
                return _ap(pc, m + part * 4 * n + 2 * n * j, [[0, 2], [n, 2], [1, m]])

            def oap(c, t):
                return _ap(u_t, c * 16 * m + t * 4 * m, [[2 * m, 2], [m, 2], [1, m]])

            for j in (0, 1):
                if SWAP_IMIM:
                    # negate-dependent imim on G so V's whole slate (rere,
                    # imre) starts right after the previous reduce
                    V.tensor_tensor(oap(0, j), lap(0, j), rap(0, j), MULT)        # +re*re
                    G.tensor_tensor(oap(0, 2 + j), nlap(j), rap(1, j), MULT)      # -im*im
                    G.tensor_tensor(oap(1, j), lap(0, j), rap(1, j), MULT)        # re*im
                    V.tensor_tensor(oap(1, 2 + j), lap(1, j), rap(0, j), MULT)    # im*re
                else:
                    V.tensor_tensor(oap(0, j), lap(0, j), rap(0, j), MULT)        # +re*re
                    V.tensor_tensor(oap(0, 2 + j), nlap(j), rap(1, j), MULT)      # -im*im
                    G.tensor_tensor(oap(1, j), lap(0, j), rap(1, j), MULT)        # re*im
                    G.tensor_tensor(oap(1, 2 + j), lap(1, j), rap(0, j), MULT)    # im*re
        else:
            # n=2: merge j into the instruction (iteration dims (i,k,j))
            def lap2(part):
                return _ap(pc, part * 4 * n, [[2 * n, 2], [0, 2], [n, 2]])

            def rap2(part):
                return _ap(pc, m + part * 4 * n, [[0, 2], [n, 2], [2 * n, 2]])

            nl2 = _ap(nim, 0, [[2, 2], [0, 2], [1, 2]])

            def oap2(c, t0):
                return _ap(u_t, c * 16 + t0 * 4, [[2, 2], [1, 2], [4, 2]])

            V.tensor_tensor(oap2(0, 0), lap2(0), rap2(0), MULT)               # +re*re
            V.tensor_tensor(oap2(0, 2), nl2, rap2(1), MULT)                   # -im*im
            G.tensor_tensor(oap2(1, 0), lap2(0), rap2(1), MULT)               # re*im
            G.tensor_tensor(oap2(1, 2), lap2(1), rap2(0), MULT)               # im*re

        if SPLIT_REDUCE:
            # re-class terms are all DVE-produced: this reduce never waits on
            # Pool, so DVE keeps streaming while Pool finishes the imag terms
            V.tensor_reduce(q[:, 0:4 * m],
                            _ap(u_t, 0, [[1, 4 * m], [4 * m, 4]]), X, ADD)
            V.tensor_reduce(q[:, 4 * m:8 * m],
                            _ap(u_t, 16 * m, [[1, 4 * m], [4 * m, 4]]), X, ADD)
        else:
            V.tensor_reduce(
                q[:, 0:8 * m],
                _ap(u_t, 0, [[16 * m, 2], [1, 4 * m], [4 * m, 4]]),
                X, ADD)
        pc = q
        n = m

    # pc: [P, 8] = Are,Bre,Cre,Dre,Aim,Bim,Cim,Dim (half-products)
    vsh = T(8, "vsh")
    mask = [16 + i for i in range(16)] + list(range(16, 32))
    V.stream_shuffle(vsh[:], pc[:], mask)

    # ZL fold into shuffled operand: R = [A,C]*ZL + [B,D]  (column 0 of V*E)
    def sv(t, base):
        return _ap(t, base, [[2, 2]])

    tb1 = T(2, "tb1")
    V.scalar_tensor_tensor(tb1[:], sv(vsh, 0), zlre[:], sv(vsh, 1), MULT, ADD)
    rre = T(2, "rre")
    V.scalar_tensor_tensor(rre[:], sv(vsh, 4), zlimn[:], tb1[:], MULT, ADD)
    tb2 = T(2, "tb2")
    V.scalar_tensor_tensor(tb2[:], sv(vsh, 0), zlim[:], sv(vsh, 5), MULT, ADD)
    # rim into rp[0:2]; rp[2:4] = -rim so one TT covers the p2&p3 blocks
    rp = T(4, "rp")
    V.scalar_tensor_tensor(rp[:, 0:2], sv(vsh, 4), zlre[:], tb2[:], MULT, ADD)
    G.tensor_scalar(rp[:, 2:4], rp[:, 0:2], -1.0, None, MULT)

    # final combine: W(i) = sum_j U(i,j)*R(j); need re/im of W(A), W(C).
    # gt groups per i: [j0p1, j1p1, j0p2, j1p2 | j0p3, j1p3, j0p4, j1p4]
    gt = T(16, "gt")
    # p1&p4: (U.re | U.im) x rre, out blocks at +0 / +6
    V.tensor_tensor(_ap(gt, 0, [[6, 2], [8, 2], [1, 2]]),
                    _ap(pc, 0, [[4, 2], [2, 2], [1, 2]]),
                    _ap(rre, 0, [[0, 2], [0, 2], [1, 2]]), MULT)
    # p2&p3: (U.im | U.re) x (-rim | rim), out blocks at +2 / +4
    G.tensor_tensor(_ap(gt, 2, [[2, 2], [8, 2], [1, 2]]),
                    _ap(pc, 4, [[-4, 2], [2, 2], [1, 2]]),
                    _ap(rp, 2, [[-2, 2], [0, 2], [1, 2]]), MULT)
    w4 = T(4, "w4")     # [WA_re, WA_im, WC_re, WC_im]
    V.tensor_reduce(w4[:], _ap(gt, 0, [[8, 2], [4, 2], [1, 4]]), X, ADD)

    sq = T(4, "sq")
    V.tensor_mul(sq[:], w4[:], w4[:])
    n2d2 = T(2, "n2d2")
    G.tensor_add(n2d2[:], _ap(sq, 0, [[2, 2]]), _ap(sq, 1, [[2, 2]]))
    d2i = T(1, "d2i")
    V.reciprocal(d2i[:], n2d2[:, 1:2])
    rat = T(1, "rat")
    V.tensor_mul(rat[:], n2d2[:, 0:1], d2i[:])
    res = T(1, "res")
    S.activation(res[:], rat[:], SQRT)

    # output DMA on ACT's queue so it never head-of-line blocks the next
    # tick's input prefetch on SP
    (nc.scalar if OUT_DMA_ON_ACT else nc.sync).dma_start(out=outd.ap(), in_=res[:])


def build_program(fpc=FPC, loop_iters=None, unroll=16):
    """SPMD program; every core evaluates its 48 freqs x 128 slices.

    loop_iters: if set, wrap the body in a pipelined hardware loop (timing
    harness only; loop_iters evaluations run with cross-iteration overlap).
    """
    nc = bacc.Bacc("TRN2", target_bir_lowering=False, debug=False)

    for cv in (-D0 / 2000.0, D0 / 2000.0, -0.5, -1.0 / 6.0, 0.5):
        th = nc.alloc_sbuf_tensor(f"cst{cv}", [128, 1], F32)
        nc.gpsimd.memset(th.ap(), cv)
        nc.const_aps.aps[(F32, cv)] = th.ap()
    nc.all_engine_barrier()

    xd = nc.dram_tensor("x", [P, 4 + 64], F32, kind="ExternalInput")
    outd = nc.dram_tensor("out", [P, 1], F32, kind="ExternalOutput")

    with tile.TileContext(nc) as tc, ExitStack() as ctx:
        pool = ctx.enter_context(tc.tile_pool(name="p", bufs=1))
        # warm the sqrt activation table once
        warm = pool.tile([P, 1], F32, name="warm", tag="warm")
        nc.scalar.activation(warm[:], nc.const_aps.aps[(F32, 1.0)][:P], SQRT)
        # t-grid is a pure constant of the model: load it once, outside the loop
        tgt = pool.tile([P, 64], F32, name="tgrid", tag="tgrid")
        nc.sync.dma_start(out=tgt[:], in_=xd.ap()[:, 4:4 + 64])
        if loop_iters is None:
            def T(w, tag, short=True):
                return pool.tile([P, w], F32, name=tag, tag=tag)
            handoff = _emit_head(nc, T, xd, tgt)
            _emit_tree_tail(nc, T, handoff, outd)
        else:
            sb = SHORT_BUFS if SHORT_BUFS else unroll

            def mkT(pipe):
                def T(w, tag, short=True):
                    b = min(sb, unroll) if short else unroll
                    return pipe.intermediate_tile([P, w], F32, name=tag, bufs=b)
                return T

            def stage1(pipe, iv):
                return _emit_head(nc, mkT(pipe), xd, tgt)

            def stage2(pipe, iv, handoff):
                _emit_tree_tail(nc, mkT(pipe), handoff, outd)

            stages = [stage1, stage2] if TWO_STAGES else [
                lambda pipe, iv: _emit_tree_tail(
                    nc, mkT(pipe),
                    _emit_head(nc, mkT(pipe), xd, tgt),
                    outd)]
            hints = tuple(getattr(mybir.EngineType, e) if isinstance(e, str)
                          else e for e in HINT_ENGINES)
            tc.For_i_pipelined(stages, 0, loop_iters, 1, pool=pool,
                               unroll=unroll, staged_num_bufs=STAGED_BUFS,
                               staggered_reset=True, hint_engines=hints)

    nc.compile()
    return nc


_PROGRAM_CACHE = {}


def _get_program(fpc):
    if fpc not in _PROGRAM_CACHE:
        _PROGRAM_CACHE[fpc] = build_program(fpc)
    return _PROGRAM_CACHE[fpc]


_REV5 = [0] * 32
for _i in range(32):
    _REV5[_i] = _bitrev5(_i)


def make_inputs(length, d1, fmin, fmax, fpc=FPC):
    """Host-side shard prep: pack [f | sqrtf | length | d1 | t-pairs] per
    partition row. Row 32q+16h+j holds freq 16q+j and slice-half h; the 64
    t-midpoints of that half are stored with pairs in bit-reversed order."""
    F = fmax - fmin
    f_full = np.arange(fmin, fmax, dtype=np.float32)
    f_pad = np.concatenate(
        [f_full, np.full(N_CORES * fpc - F, float(fmin), np.float32)])
    t = ((np.arange(N_SUB, dtype=np.float32) + 0.5) / N_SUB)
    # t-pair columns for each half, bit-reversed pair order
    tcols = np.empty((2, 64), np.float32)
    for h in (0, 1):
        for i in range(32):
            p = _REV5[i]
            tcols[h, 2 * i] = t[64 * h + 2 * p]
            tcols[h, 2 * i + 1] = t[64 * h + 2 * p + 1]
    in_maps = []
    for c in range(N_CORES):
        Xa = np.zeros((P, 4 + 64), dtype=np.float32)
        for q in range(3):
            for hh in (0, 1):
                rows = slice(32 * q + 16 * hh, 32 * q + 16 * hh + 16)
                fi = f_pad[c * fpc + 16 * q:c * fpc + 16 * q + 16]
                Xa[rows, 0] = fi
                Xa[rows, 1] = np.sqrt(fi)
                Xa[rows, 2] = np.float32(length[0])
                Xa[rows, 3] = np.float32(d1[0])
                Xa[rows, 4:] = tcols[hh][None, :]
        in_maps.append({"x": Xa})
    return in_maps


def gather_outputs(res_list, fmin, fmax, fpc=FPC):
    F = fmax - fmin
    out = np.empty(N_CORES * fpc, np.float32)
    for c in range(N_CORES):
        r = res_list[c].reshape(-1)
        for q in range(3):
            out[c * fpc + 16 * q:c * fpc + 16 * q + 16] = r[32 * q:32 * q + 16]
    return out[:F]


def kernel(length, d1, fmin, fmax):
    length = np.asarray(length, dtype=np.float32)
    d1 = np.asarray(d1, dtype=np.float32)
    fmin = int(fmin)
    fmax = int(fmax)
    F = fmax - fmin
    assert F <= N_CORES * FPC, "frequency range exceeds hardcoded capacity"
    nc = _get_program(FPC)
    in_maps = make_inputs(length, d1, fmin, fmax, FPC)
    res = run_bass_kernel_spmd(nc, in_maps, list(range(N_CORES)))
    outs = [res.results[c]["out"] for c in range(N_CORES)]
    return gather_outputs(outs, fmin, fmax, FPC).astype(np.float32)
